# revision 1
# baseline (speedup 1.0000x reference)
"""Grouped-Query Attention on 8 Trainium2 NeuronCores.

Sharding: TP-4 over KV groups x DP-2 over batch.
Core c handles batch b = c // 4, group g = c % 4 (4 query heads, 1 KV group).
Each core computes q/k/v projections for its heads, causal attention, and a
partial O-projection (its 512 input columns of Wo); the host sums the 4 TP
partials per batch and adds bo.

All matmuls run in bf16 with fp32 PSUM accumulation.  Layout is fully
"transposed" on device so no on-chip transposes of activations are needed:
  qT, kT: [d=128 partitions, t]        (proj computed as W^T @ x^T)
  S^T tiles: [tk=128, tq=512] = kT_blk.T @ qT   (one matmul each)
  E = exp(S^T * scale), causal-masked via precomputed 0/1 tiles
  row-sums of softmax = ones^T @ E (PE), broadcast via rank-1 matmul
  attn^T [d, tq] = V^T @ E accumulated over tk blocks (V natural [tk, d])
  out [tq, e] partial = attn^T.T @ Wo_rows accumulated over the 4 heads
"""

import numpy as np
import ml_dtypes

EMBED = 2048
T = 2048
D = 128           # head dim
NQH = 16          # query heads
NG = 4            # kv groups
HPG = NQH // NG   # query heads per group = 4
NCORES = 8
ECH = EMBED // 128   # 16 contraction chunks
TC = T // 512        # 4 t-chunks of 512
TT = T // 128        # 16 t-tiles of 128
SCALE = 1.0 / float(np.sqrt(D))

_PROG = {}


def build_program():
    if "nc" in _PROG:
        return _PROG["nc"]

    from contextlib import ExitStack
    import concourse.mybir as mybir
    from concourse import bacc, tile
    from concourse.masks import make_identity

    # The Tile legalizer emits one Ldweights per Matmult even when consecutive
    # matmuls reuse the same stationary operand; the PE sequencer cost of the
    # redundant loads is significant.  Wrap tile_legalize to drop an Ldweights
    # whose key (weights AP + mode) matches the immediately preceding one.
    if not getattr(tile.tile_legalize, "_ldw_dedup", False):
        _orig_legalize = tile.tile_legalize

        def _dedup_legalize(ordered, nc_):
            ordered = _orig_legalize(ordered, nc_)
            dropped = 0
            for bb, insts in ordered.items():
                out = []
                state = None
                for inst in insts:
                    tn = type(inst).__name__
                    if tn == "InstLdweights":
                        key = (
                            str(inst.ins[0]),
                            str(getattr(inst, "is_transpose", None)),
                            str(getattr(inst, "tile_position", None)),
                            str(getattr(inst, "perf_mode", None)),
                        )
                        if key == state:
                            dropped += 1
                            continue
                        state = key
                    out.append(inst)
                ordered[bb] = out
            return ordered

        _dedup_legalize._ldw_dedup = True
        tile.tile_legalize = _dedup_legalize

    dt = mybir.dt
    BF = dt.bfloat16
    F32 = dt.float32
    AF = mybir.ActivationFunctionType

    nc = bacc.Bacc("TRN2", target_bir_lowering=False, debug=False)

    xt_d = nc.dram_tensor("xt", [ECH, 128, T], BF, kind="ExternalInput")
    wq_d = nc.dram_tensor("wq", [ECH, 128, HPG * D], BF, kind="ExternalInput")
    wk_d = nc.dram_tensor("wk", [ECH, 128, D], BF, kind="ExternalInput")
    wv_d = nc.dram_tensor("wv", [ECH, 128, D], BF, kind="ExternalInput")
    wo_d = nc.dram_tensor("wo", [HPG, 128, EMBED], BF, kind="ExternalInput")
    cm_d = nc.dram_tensor("cmask", [4, 128, 512], BF, kind="ExternalInput")
    bq_d = nc.dram_tensor("bq", [128, HPG], F32, kind="ExternalInput")
    bk_d = nc.dram_tensor("bk", [128, 1], F32, kind="ExternalInput")
    bv_d = nc.dram_tensor("bv", [128, 1], F32, kind="ExternalInput")
    out_d = nc.dram_tensor("out", [T, EMBED], F32, kind="ExternalOutput")

    with tile.TileContext(nc) as tc, ExitStack() as ctx:
        pers = ctx.enter_context(tc.tile_pool(name="pers", bufs=1))

        wq_sb = pers.tile([128, ECH, HPG * D], BF)
        wk_sb = pers.tile([128, ECH, D], BF)
        wv_sb = pers.tile([128, ECH, D], BF)
        wo_sb = pers.tile([128, HPG, EMBED], BF)
        cm_sb = pers.tile([128, 4, 512], BF)
        bq_sb = pers.tile([128, HPG], F32)
        bk_sb = pers.tile([128, 1], F32)
        bv_sb = pers.tile([128, 1], F32)
        qT_sb = pers.tile([128, HPG, T], BF)
        kT_sb = pers.tile([128, T], BF)
        vT_sb = pers.tile([128, T], BF)
        v_sb = pers.tile([128, TT, D], BF)
        ones_col = pers.tile([128, 1], BF)
        ones_row = pers.tile([1, 128], BF)
        ident = pers.tile([128, 128], BF)

        nc.gpsimd.memset(ones_col[:], 1.0)
        nc.gpsimd.memset(ones_row[:], 1.0)
        make_identity(nc, ident[:])

        # weights stream on the ACT DMA queue so they load in parallel with
        # the xt stream on the sync queue (PE starts once xt[0]+wq[0] land);
        # batched into one transfer each to keep ACT's sequencer free for the
        # PSUM->SBUF copies.
        nc.scalar.dma_start(wq_sb[:], wq_d.ap().rearrange("e p c -> p e c"))
        nc.scalar.dma_start(wk_sb[:], wk_d.ap().rearrange("e p c -> p e c"))
        nc.scalar.dma_start(wv_sb[:], wv_d.ap().rearrange("e p c -> p e c"))
        nc.scalar.dma_start(bq_sb[:], bq_d[:])
        nc.scalar.dma_start(bk_sb[:], bk_d[:])
        nc.scalar.dma_start(bv_sb[:], bv_d[:])
        nc.scalar.dma_start(wo_sb[:], wo_d.ap().rearrange("h p e -> p h e"))
        nc.scalar.dma_start(cm_sb[:], cm_d.ap().rearrange("j p c -> p j c"))

        # ---- Phase 1: projections (transposed: qT/kT/vT = W_blk^T @ x^T) ----
        with (
            tc.tile_pool(name="xtp", bufs=1) as xtp,
            tc.tile_pool(name="pp", bufs=2, space="PSUM") as pp,
        ):
            xt_sb = xtp.tile([128, ECH, T], BF)
            for ec in range(ECH):
                nc.sync.dma_start(xt_sb[:, ec, :], xt_d[ec])

            for j in range(HPG + 2):  # 4 q heads, then k, then v
                ps = pp.tile([128, T], F32, tag="pp")
                for ec in range(ECH):
                    if j < HPG:
                        lhsT = wq_sb[:, ec, j * D:(j + 1) * D]
                    elif j == HPG:
                        lhsT = wk_sb[:, ec, :]
                    else:
                        lhsT = wv_sb[:, ec, :]
                    for t5 in range(TC):
                        nc.tensor.matmul(
                            ps[:, t5 * 512:(t5 + 1) * 512],
                            lhsT,
                            xt_sb[:, ec, t5 * 512:(t5 + 1) * 512],
                            start=(ec == 0),
                            stop=(ec == ECH - 1),
                        )
                for t5 in range(TC):
                    sl = slice(t5 * 512, (t5 + 1) * 512)
                    if j < HPG:
                        nc.scalar.activation(
                            qT_sb[:, j, sl], ps[:, sl], AF.Identity,
                            bias=bq_sb[:, j:j + 1],
                        )
                    elif j == HPG:
                        nc.scalar.activation(
                            kT_sb[:, sl], ps[:, sl], AF.Identity, bias=bk_sb[:]
                        )
                    else:
                        nc.scalar.activation(
                            vT_sb[:, sl], ps[:, sl], AF.Identity, bias=bv_sb[:]
                        )

        # ---- v natural layout via PE transposes ----
        with tc.tile_pool(name="pt", bufs=2, space="PSUM") as pt:
            for tt in range(TT):
                ptile = pt.tile([128, D], BF, tag="pt")
                nc.tensor.transpose(ptile[:], vT_sb[:, tt * D:(tt + 1) * D], ident[:])
                nc.vector.tensor_copy(v_sb[:, tt, :], ptile[:])

        # ---- Phase 2/3: attention + O-projection ----
        with (
            tc.tile_pool(name="eb", bufs=2) as ebp,
            tc.tile_pool(name="ntp", bufs=2) as ntp,
            tc.tile_pool(name="rcp", bufs=2) as rcp,
            tc.tile_pool(name="sms", bufs=2) as smp,
            tc.tile_pool(name="fsb", bufs=3) as fsb,
            tc.tile_pool(name="ps2", bufs=2, space="PSUM") as ps2,
            tc.tile_pool(name="ps1", bufs=1, space="PSUM") as ps1,
            tc.tile_pool(name="pso", bufs=1, space="PSUM") as pso,
            tc.tile_pool(name="psf", bufs=1, space="PSUM") as psf,
        ):
            for qc in range(TC):
                nk = 4 * (qc + 1)  # causal: tk blocks 0..nk-1
                nT = ntp.tile([128, HPG, 512], BF, tag="nt")
                for h in range(HPG):
                    E = ebp.tile([128, nk, 512], BF, tag="E")
                    for tkp in range(nk // 2):
                        s2 = ps2.tile([128, 2, 512], F32, tag="s2")
                        for u in range(2):
                            tk = tkp * 2 + u
                            nc.tensor.matmul(
                                s2[:, u, :],
                                kT_sb[:, tk * D:(tk + 1) * D],
                                qT_sb[:, h, qc * 512:(qc + 1) * 512],
                                start=True,
                                stop=True,
                            )
                        nc.scalar.activation(
                            E[:, tkp * 2:tkp * 2 + 2, :], s2[:], AF.Exp, scale=SCALE
                        )
                        for u in range(2):
                            tk = tkp * 2 + u
                            if tk >= 4 * qc:
                                nc.vector.tensor_mul(
                                    E[:, tk, :], E[:, tk, :], cm_sb[:, tk - 4 * qc, :]
                                )
                    # softmax denominators: ones^T @ E accumulated over tk
                    psum = ps1.tile([1, 512], F32, tag="psum")
                    for tk in range(nk):
                        nc.tensor.matmul(
                            psum[:], ones_col[:], E[:, tk, :],
                            start=(tk == 0), stop=(tk == nk - 1),
                        )
                    sums_bf = smp.tile([1, 512], BF, tag="sums")
                    nc.vector.tensor_copy(sums_bf[:], psum[:])
                    sumsB = ps2.tile([128, 512], F32, tag="s2")
                    nc.tensor.matmul(
                        sumsB[:], ones_row[:], sums_bf[:], start=True, stop=True
                    )
                    recipB = rcp.tile([128, 512], F32, tag="recip")
                    nc.vector.reciprocal(recipB[:], sumsB[:])
                    # attn^T = V^T @ E accumulated over tk blocks
                    po = pso.tile([128, 512], F32, tag="po")
                    for tk in range(nk):
                        nc.tensor.matmul(
                            po[:], v_sb[:, tk, :], E[:, tk, :],
                            start=(tk == 0), stop=(tk == nk - 1),
                        )
                    nc.vector.tensor_mul(nT[:, h, :], po[:], recipB[:])
                # O-projection for this q-chunk (partial over this core's 512 cols)
                for qt in range(4):
                    row = qc * 4 + qt
                    for ech in range(2):
                        pf = psf.tile([128, 2, 512], F32, tag="pf")
                        for h in range(HPG):
                            lhsT = nT[:, h, qt * 128:(qt + 1) * 128]
                            for e2 in range(2):
                                ecol = ech * 2 + e2
                                nc.tensor.matmul(
                                    pf[:, e2, :],
                                    lhsT,
                                    wo_sb[:, h, ecol * 512:(ecol + 1) * 512],
                                    start=(h == 0),
                                    stop=(h == HPG - 1),
                                )
                        for e2 in range(2):
                            ecol = ech * 2 + e2
                            f_t = fsb.tile([128, 512], F32, tag="f")
                            nc.vector.tensor_copy(f_t[:], pf[:, e2, :])
                            nc.sync.dma_start(
                                out_d[row * 128:(row + 1) * 128,
                                      ecol * 512:(ecol + 1) * 512],
                                f_t[:],
                            )

    nc.compile()
    _PROG["nc"] = nc
    return nc


def prepare_in_maps(x, Wq, bq, Wk, bk, Wv, bv, Wo, bo):
    bf = ml_dtypes.bfloat16
    # causal mask tiles for the 4 diagonal tk offsets (S^T layout):
    # tile j, element (p, f) is valid iff tk = j*128 + p <= f  (f = q offset)
    p = np.arange(128)[:, None]
    f = np.arange(512)[None, :]
    cmask = np.stack(
        [(f >= j * 128 + p).astype(bf) for j in range(4)], axis=0
    )  # [4,128,512]

    in_maps = []
    for c in range(NCORES):
        b, g = c // 4, c % 4
        xt = x[b].T.astype(bf).reshape(ECH, 128, T)
        wq = np.ascontiguousarray(Wq[:, g * 512:(g + 1) * 512]).astype(bf).reshape(
            ECH, 128, HPG * D
        )
        wk = np.ascontiguousarray(Wk[:, g * D:(g + 1) * D]).astype(bf).reshape(
            ECH, 128, D
        )
        wv = np.ascontiguousarray(Wv[:, g * D:(g + 1) * D]).astype(bf).reshape(
            ECH, 128, D
        )
        wo = np.ascontiguousarray(Wo[g * 512:(g + 1) * 512, :]).astype(bf).reshape(
            HPG, 128, EMBED
        )
        bqc = np.ascontiguousarray(
            bq[g * 512:(g + 1) * 512].reshape(HPG, 128).T
        ).astype(np.float32)
        bkc = bk[g * D:(g + 1) * D].reshape(128, 1).astype(np.float32)
        bvc = bv[g * D:(g + 1) * D].reshape(128, 1).astype(np.float32)
        in_maps.append(
            {
                "xt": xt,
                "wq": wq,
                "wk": wk,
                "wv": wv,
                "wo": wo,
                "cmask": cmask,
                "bq": bqc,
                "bk": bkc,
                "bv": bvc,
            }
        )
    return in_maps


def combine_outputs(results, bo):
    out = np.empty((2, T, EMBED), dtype=np.float32)
    for b in range(2):
        acc = results[b * 4]["out"].copy()
        for g in range(1, 4):
            acc += results[b * 4 + g]["out"]
        out[b] = acc + bo[None, :].astype(np.float32)
    return out


def kernel(x, Wq, bq, Wk, bk, Wv, bv, Wo, bo):
    from concourse.bass_utils import run_bass_kernel_spmd

    nc = build_program()
    in_maps = prepare_in_maps(x, Wq, bq, Wk, bk, Wv, bv, Wo, bo)
    res = run_bass_kernel_spmd(nc, in_maps, list(range(NCORES)))
    return combine_outputs(res.results, np.asarray(bo))



# revision 2
# speedup vs baseline: 1.1844x; 1.1844x over previous
"""Grouped-Query Attention on 8 Trainium2 NeuronCores.

Sharding: TP-4 over KV groups x DP-2 over batch.
Core c handles batch b = c // 4, group g = c % 4 (4 query heads, 1 KV group).
Each core computes q/k/v projections for its heads, causal attention, and a
partial O-projection (its 512 input columns of Wo); the host sums the 4 TP
partials per batch and adds bo.

All matmuls run in bf16 with fp32 PSUM accumulation.  Layout is fully
"transposed" on device so no on-chip transposes of activations are needed:
  qT, kT: [d=128 partitions, t]        (proj computed as W^T @ x^T)
  S^T tiles: [tk=128, <=512 q] = kT_blk.T @ qT, exact-causal column ranges
  E = exp(S^T * scale); the 128x128 diagonal triangle is masked on GPSIMD
  row-sums of softmax = ones128^T @ E (PE) -- fused sum+broadcast
  attn^T [d, tq] = V^T @ E accumulated over tk blocks (V natural [tk, d])
  out [tq, e] partial = attn^T.T @ Wo_rows accumulated over the 4 heads

The attention inner loop is software-pipelined one head deep: while head h's
S tiles stream through PE/ACT, head h-1's rowsum+AV matmuls interleave on PE
so PE never waits on the (slower) exp stream.
"""

import numpy as np
import ml_dtypes

EMBED = 2048
T = 2048
D = 128           # head dim
NQH = 16          # query heads
NG = 4            # kv groups
HPG = NQH // NG   # query heads per group = 4
NCORES = 8
ECH = EMBED // 128   # 16 contraction chunks
TC = T // 512        # 4 q-chunks of 512
TT = T // 128        # 16 t-tiles of 128
SCALE = 1.0 / float(np.sqrt(D))

_PROG = {}


def build_program():
    if "nc" in _PROG:
        return _PROG["nc"]

    from contextlib import ExitStack
    import concourse.mybir as mybir
    from concourse import bacc, tile
    from concourse.masks import make_identity

    # The Tile legalizer emits one Ldweights per Matmult even when consecutive
    # matmuls reuse the same stationary operand; the PE sequencer cost of the
    # redundant loads is significant.  Wrap tile_legalize to drop an Ldweights
    # whose key (weights AP + mode) matches the immediately preceding one.
    if not getattr(tile.tile_legalize, "_ldw_dedup", False):
        _orig_legalize = tile.tile_legalize

        def _dedup_legalize(ordered, nc_):
            ordered = _orig_legalize(ordered, nc_)
            dropped = 0
            for bb, insts in ordered.items():
                out = []
                state = None
                for inst in insts:
                    tn = type(inst).__name__
                    if tn == "InstLdweights":
                        key = (
                            str(inst.ins[0]),
                            str(getattr(inst, "is_transpose", None)),
                            str(getattr(inst, "tile_position", None)),
                            str(getattr(inst, "perf_mode", None)),
                        )
                        if key == state:
                            dropped += 1
                            continue
                        state = key
                    out.append(inst)
                ordered[bb] = out
            return ordered

        _dedup_legalize._ldw_dedup = True
        tile.tile_legalize = _dedup_legalize

    dt = mybir.dt
    BF = dt.bfloat16
    F32 = dt.float32
    AF = mybir.ActivationFunctionType

    nc = bacc.Bacc("TRN2", target_bir_lowering=False, debug=False)

    xt_d = nc.dram_tensor("xt", [ECH, 128, T], BF, kind="ExternalInput")
    wq_d = nc.dram_tensor("wq", [ECH, 128, HPG * D], BF, kind="ExternalInput")
    wk_d = nc.dram_tensor("wk", [ECH, 128, D], BF, kind="ExternalInput")
    wv_d = nc.dram_tensor("wv", [ECH, 128, D], BF, kind="ExternalInput")
    wo_d = nc.dram_tensor("wo", [HPG, 128, EMBED], BF, kind="ExternalInput")
    tril_d = nc.dram_tensor("tril", [128, 128], BF, kind="ExternalInput")
    bq_d = nc.dram_tensor("bq", [128, HPG], F32, kind="ExternalInput")
    bk_d = nc.dram_tensor("bk", [128, 1], F32, kind="ExternalInput")
    bv_d = nc.dram_tensor("bv", [128, 1], F32, kind="ExternalInput")
    out_d = nc.dram_tensor("out", [T, EMBED], BF, kind="ExternalOutput")

    with tile.TileContext(nc) as tc, ExitStack() as ctx:
        pers = ctx.enter_context(tc.tile_pool(name="pers", bufs=1))

        wq_sb = pers.tile([128, ECH, HPG * D], BF)
        wk_sb = pers.tile([128, ECH, D], BF)
        wv_sb = pers.tile([128, ECH, D], BF)
        wo_sb = pers.tile([128, HPG, EMBED], BF)
        tril_sb = pers.tile([128, 128], BF)
        bq_sb = pers.tile([128, HPG], F32)
        bk_sb = pers.tile([128, 1], F32)
        bv_sb = pers.tile([128, 1], F32)
        qT_sb = pers.tile([128, HPG, T], BF)
        kT_sb = pers.tile([128, T], BF)
        vT_sb = pers.tile([128, T], BF)
        v_sb = pers.tile([128, TT, D], BF)
        ones128 = pers.tile([128, 128], BF)
        ident = pers.tile([128, 128], BF)

        nc.gpsimd.memset(ones128[:], 1.0)
        make_identity(nc, ident[:])

        # small weights needed first (k/v projections lead) go out first;
        # the big wq/wo transfers are emitted later from inside phase 1 so
        # they don't contend with the xt stream on the DMA engines.
        nc.scalar.dma_start(wk_sb[:], wk_d.ap().rearrange("e p c -> p e c"))
        nc.scalar.dma_start(wv_sb[:], wv_d.ap().rearrange("e p c -> p e c"))
        nc.scalar.dma_start(bk_sb[:], bk_d[:])
        nc.scalar.dma_start(bv_sb[:], bv_d[:])
        nc.scalar.dma_start(bq_sb[:], bq_d[:])
        nc.scalar.dma_start(tril_sb[:], tril_d[:])

        # ---- Phase 1: projections (transposed: qT/kT/vT = W_blk^T @ x^T) ----
        with (
            tc.tile_pool(name="xtp", bufs=1) as xtp,
            tc.tile_pool(name="pp", bufs=2, space="PSUM") as pp,
        ):
            xt_sb = xtp.tile([128, ECH, T], BF)
            for ec in range(ECH):
                nc.sync.dma_start(xt_sb[:, ec, :], xt_d[ec])

            # j: 0 = k, 1 = v, 2..5 = q heads 0..3
            for j in range(HPG + 2):
                ps = pp.tile([128, T], F32, tag="pp")
                for ec in range(ECH):
                    if j == 0:
                        lhsT = wk_sb[:, ec, :]
                    elif j == 1:
                        lhsT = wv_sb[:, ec, :]
                    else:
                        h = j - 2
                        lhsT = wq_sb[:, ec, h * D:(h + 1) * D]
                    for t5 in range(TC):
                        nc.tensor.matmul(
                            ps[:, t5 * 512:(t5 + 1) * 512],
                            lhsT,
                            xt_sb[:, ec, t5 * 512:(t5 + 1) * 512],
                            start=(ec == 0),
                            stop=(ec == ECH - 1),
                        )
                for t5 in range(TC):
                    sl = slice(t5 * 512, (t5 + 1) * 512)
                    if j == 0:
                        nc.scalar.activation(
                            kT_sb[:, sl], ps[:, sl], AF.Identity, bias=bk_sb[:]
                        )
                    elif j == 1:
                        nc.scalar.activation(
                            vT_sb[:, sl], ps[:, sl], AF.Identity, bias=bv_sb[:]
                        )
                    else:
                        h = j - 2
                        nc.scalar.activation(
                            qT_sb[:, h, sl], ps[:, sl], AF.Identity,
                            bias=bq_sb[:, h:h + 1],
                        )
                # stagger the big weight loads behind the early drains
                if j == 0:
                    nc.scalar.dma_start(
                        wq_sb[:], wq_d.ap().rearrange("e p c -> p e c")
                    )
                elif j == 1:
                    nc.scalar.dma_start(
                        wo_sb[:], wo_d.ap().rearrange("h p e -> p h e")
                    )

        # ---- v natural layout via PE transposes ----
        with tc.tile_pool(name="pt", bufs=2, space="PSUM") as pt:
            for tt in range(TT):
                ptile = pt.tile([128, D], BF, tag="pt")
                nc.tensor.transpose(ptile[:], vT_sb[:, tt * D:(tt + 1) * D], ident[:])
                nc.vector.tensor_copy(v_sb[:, tt, :], ptile[:])

        # ---- Phase 2/3: attention + O-projection, head-lag pipelined ----
        with (
            tc.tile_pool(name="eb", bufs=2) as ebp,
            tc.tile_pool(name="ntp", bufs=2) as ntp,
            tc.tile_pool(name="rcp", bufs=2) as rcp,
            tc.tile_pool(name="fsb", bufs=4) as fsb,
            tc.tile_pool(name="ps2", bufs=2, space="PSUM") as ps2,
            tc.tile_pool(name="psr", bufs=2, space="PSUM") as psr,
            tc.tile_pool(name="pso", bufs=2, space="PSUM") as pso,
            tc.tile_pool(name="psf", bufs=2, space="PSUM") as psf,
        ):
            slots = [(qc, h) for qc in range(TC) for h in range(HPG)]

            def s_work(E, qc, h, t):
                """S^T tile t for (qc, h): matmul + exp (+ triangle mask)."""
                off = 128 * max(0, t - 4 * qc)
                s2 = ps2.tile([128, 512], F32, tag="s2")
                nc.tensor.matmul(
                    s2[:, off:512],
                    kT_sb[:, t * D:(t + 1) * D],
                    qT_sb[:, h, qc * 512 + off:(qc + 1) * 512],
                    start=True,
                    stop=True,
                )
                nc.scalar.activation(
                    E[:, t, off:512], s2[:, off:512], AF.Exp, scale=SCALE
                )
                if t >= 4 * qc:
                    nc.gpsimd.tensor_mul(
                        E[:, t, off:off + 128], E[:, t, off:off + 128], tril_sb[:]
                    )

            def finish_head(E, rs, po, nT, qc, h):
                """reciprocal of rowsums + normalize AV output into nT."""
                recipB = rcp.tile([128, 512], F32, tag="recip")
                nc.vector.reciprocal(recipB[:], rs[:])
                nc.vector.tensor_mul(nT[:, h, :], po[:], recipB[:])

            def o_proj(nT, qc):
                """partial out[qc-chunk] = attn^T.T @ Wo_rows, acc over heads."""
                for qt in range(4):
                    row = qc * 4 + qt
                    for ecol in range(4):
                        pf = psf.tile([128, 512], F32, tag="pf")
                        for h in range(HPG):
                            nc.tensor.matmul(
                                pf[:],
                                nT[:, h, qt * 128:(qt + 1) * 128],
                                wo_sb[:, h, ecol * 512:(ecol + 1) * 512],
                                start=(h == 0),
                                stop=(h == HPG - 1),
                            )
                        f_t = fsb.tile([128, 512], BF, tag="f")
                        if ecol % 2 == 0:
                            nc.vector.tensor_copy(f_t[:], pf[:])
                        else:
                            nc.scalar.activation(f_t[:], pf[:], AF.Identity)
                        nc.sync.dma_start(
                            out_d[row * 128:(row + 1) * 128,
                                  ecol * 512:(ecol + 1) * 512],
                            f_t[:],
                        )

            prev = None  # (E, rs, po, nT, qc, h) of the in-flight head
            nT = None
            for qc, h in slots:
                nk = 4 * (qc + 1)
                if h == 0:
                    nT = ntp.tile([128, HPG, 512], BF, tag="nt")
                E = ebp.tile([128, TT, 512], BF, tag="E")
                rs = psr.tile([128, 512], F32, tag="rs")
                po = pso.tile([128, 512], F32, tag="po")
                pnk = 4 * (prev[4] + 1) if prev is not None else 0
                for t in range(nk):
                    s_work(E, qc, h, t)
                    if prev is not None and t < pnk:
                        pE, prs, ppo, pnT, pqc, ph = prev
                        poff = 128 * max(0, t - 4 * pqc)
                        nc.tensor.matmul(
                            prs[:, poff:512], ones128[:], pE[:, t, poff:512],
                            start=(t == 0), stop=(t == pnk - 1),
                        )
                        nc.tensor.matmul(
                            ppo[:, poff:512], v_sb[:, t, :], pE[:, t, poff:512],
                            start=(t == 0), stop=(t == pnk - 1),
                        )
                if prev is not None:
                    pE, prs, ppo, pnT, pqc, ph = prev
                    finish_head(pE, prs, ppo, pnT, pqc, ph)
                    if ph == HPG - 1:
                        o_proj(pnT, pqc)
                prev = (E, rs, po, nT, qc, h)

            # drain the last head and the final O-projection
            E, rs, po, nT, qc, h = prev
            nk = 4 * (qc + 1)
            for t in range(nk):
                off = 128 * max(0, t - 4 * qc)
                nc.tensor.matmul(
                    rs[:, off:512], ones128[:], E[:, t, off:512],
                    start=(t == 0), stop=(t == nk - 1),
                )
                nc.tensor.matmul(
                    po[:, off:512], v_sb[:, t, :], E[:, t, off:512],
                    start=(t == 0), stop=(t == nk - 1),
                )
            finish_head(E, rs, po, nT, qc, h)
            o_proj(nT, qc)

    nc.compile()
    _PROG["nc"] = nc
    return nc


def prepare_in_maps(x, Wq, bq, Wk, bk, Wv, bv, Wo, bo):
    bf = ml_dtypes.bfloat16
    # lower-triangle mask for the 128x128 diagonal tile of S^T:
    # element (p, f) is valid iff k offset p <= q offset f
    p = np.arange(128)[:, None]
    f = np.arange(128)[None, :]
    tril = (f >= p).astype(bf)

    in_maps = []
    for c in range(NCORES):
        b, g = c // 4, c % 4
        xt = x[b].T.astype(bf).reshape(ECH, 128, T)
        wq = np.ascontiguousarray(Wq[:, g * 512:(g + 1) * 512]).astype(bf).reshape(
            ECH, 128, HPG * D
        )
        wk = np.ascontiguousarray(Wk[:, g * D:(g + 1) * D]).astype(bf).reshape(
            ECH, 128, D
        )
        wv = np.ascontiguousarray(Wv[:, g * D:(g + 1) * D]).astype(bf).reshape(
            ECH, 128, D
        )
        wo = np.ascontiguousarray(Wo[g * 512:(g + 1) * 512, :]).astype(bf).reshape(
            HPG, 128, EMBED
        )
        bqc = np.ascontiguousarray(
            bq[g * 512:(g + 1) * 512].reshape(HPG, 128).T
        ).astype(np.float32)
        bkc = bk[g * D:(g + 1) * D].reshape(128, 1).astype(np.float32)
        bvc = bv[g * D:(g + 1) * D].reshape(128, 1).astype(np.float32)
        in_maps.append(
            {
                "xt": xt,
                "wq": wq,
                "wk": wk,
                "wv": wv,
                "wo": wo,
                "tril": tril,
                "bq": bqc,
                "bk": bkc,
                "bv": bvc,
            }
        )
    return in_maps


def combine_outputs(results, bo):
    out = np.empty((2, T, EMBED), dtype=np.float32)
    for b in range(2):
        acc = results[b * 4]["out"].astype(np.float32)
        for g in range(1, 4):
            acc += results[b * 4 + g]["out"].astype(np.float32)
        out[b] = acc + bo[None, :].astype(np.float32)
    return out


def kernel(x, Wq, bq, Wk, bk, Wv, bv, Wo, bo):
    from concourse.bass_utils import run_bass_kernel_spmd

    nc = build_program()
    in_maps = prepare_in_maps(x, Wq, bq, Wk, bk, Wv, bv, Wo, bo)
    res = run_bass_kernel_spmd(nc, in_maps, list(range(NCORES)))
    return combine_outputs(res.results, np.asarray(bo))
